# revision 38
# baseline (speedup 1.0000x reference)
"""Trainium2 Bass kernel for the augmented-ODE-RHS (primal + 4 JVPs) problem.

Math (per sample; w=omega, v=omega_dot, K=(k0..k3), aug pairs (a_p, b_p)):
    mM = k0*w + k1*v            M = 10 - mM        A = 1/M
    mD = k2*w + k3*v            E = mD - 1 (= -D)
    u  = 0.2*w + v
    g  = 0.02 - 4*w + E*u       P = A*g
    f2 = P - 0.2*v
    out[0] = v, out[1] = f2
JVP p (tangent (a_p, b_p, e_p)) collapses to a per-sample linear form:
    alpha = -4A + 0.2*A*E + (A*u)*k2 + (A*P)*k0
    beta  = A*E - 0.2 + (A*u)*k3 + (A*P)*k1
    gamma_p in (A*P*w, A*P*v, A*u*w, A*u*v)
    out[2+2p] = b_p,  out[3+2p] = alpha*a_p + beta*b_p + gamma_p

Sharding: pure data parallel over the batch across 8 NeuronCores. Each core
gets R = 128*CHUNKS*N rows (inputs zero-padded up to 8R). Per core, rows are
laid out so SBUF partition j owns a contiguous slab of rows -> every DMA is
128 fully-contiguous multi-KB segments.

Engine split per chunk (fp32): DVE does the tensor*tensor products (30N),
GPSIMD the pure adds (14N), ACT the affine/copies (9N), HWDGE the DMAs.
"""

import json

import numpy as np

N_CORES = 8
P = 128
CHUNKS = 10

_CACHE: dict = {}


def _fix_bir_json(raw: bytes) -> bytes:
    """Walrus in this container encodes at most ONE sem-wait and ONE sem-update
    per instruction. Tile attaches several. Split the extras onto single-wait /
    single-update EventSemaphore instructions on the same engine, placed just
    before (waits) / after (updates) the original — identical sync semantics."""
    m = json.loads(raw)
    ctr = 0
    for fn in m["functions"]:
        for blk in fn["blocks"]:
            out = []
            for ins in blk["instructions"]:
                si = ins.get("sync_info")
                pend_updates = []
                if si:
                    waits = si.get("on_wait") or []
                    if len(waits) > 1:
                        for w in waits[:-1]:
                            ctr += 1
                            ev = {
                                "engine": ins["engine"], "ins": [], "outs": [],
                                "name": f"xw-{ctr}",
                                "opcode": "EventSemaphore",
                                "sync_info": {"on_update": [], "on_wait": [w]},
                            }
                            if "debug" in ins:
                                ev["debug"] = ins["debug"]
                            out.append(ev)
                        si["on_wait"] = [waits[-1]]
                    ups = si.get("on_update") or []
                    if len(ups) > 1:
                        assert ins.get("opcode") != "DMACopy", \
                            "DMACopy with >1 sem updates cannot be split"
                        si["on_update"] = [ups[0]]
                        pend_updates = ups[1:]
                out.append(ins)
                for u in pend_updates:
                    ctr += 1
                    ev = {
                        "engine": ins["engine"], "ins": [], "outs": [],
                        "name": f"xu-{ctr}",
                        "opcode": "EventSemaphore",
                        "sync_info": {"on_update": [u], "on_wait": []},
                    }
                    if "debug" in ins:
                        ev["debug"] = ins["debug"]
                    out.append(ev)
            blk["instructions"] = out
    return json.dumps(m).encode()


def _build(R: int, N: int, reps: int = 1):
    import concourse.bass as bass
    import concourse.tile as tile
    import concourse.mybir as mybir

    F32 = mybir.dt.float32
    mul = mybir.AluOpType.mult
    add = mybir.AluOpType.add
    Copy = mybir.ActivationFunctionType.Copy

    nc = bass.Bass("TRN2")

    state_d = nc.dram_tensor("state", [R, 10], F32, kind="ExternalInput")
    k_d = nc.dram_tensor("K", [R, 4], F32, kind="ExternalInput")
    out_d = nc.dram_tensor("out", [R, 10], F32, kind="ExternalOutput")

    sv = state_d[:].rearrange("(p n) m -> p (n m)", p=P)
    kv = k_d[:].rearrange("(p n) m -> p (n m)", p=P)
    ov = out_d[:].rearrange("(p n) m -> p (n m)", p=P)

    with tile.TileContext(nc) as tc:
        with (
            tc.tile_pool(name="io", bufs=2) as io,
            tc.tile_pool(name="tmp", bufs=1) as tp,
            tc.tile_pool(name="tmp2", bufs=2) as tp2,
        ):
            for c in [c for _ in range(reps) for c in range(CHUNKS)]:
                S_t = io.tile([P, 10 * N], F32, tag="S")
                K_t = io.tile([P, 4 * N], F32, tag="K")
                O_t = io.tile([P, 10 * N], F32, tag="O")
                nc.sync.dma_start(S_t[:], sv[:, c * 10 * N:(c + 1) * 10 * N])
                nc.sync.dma_start(K_t[:], kv[:, c * 4 * N:(c + 1) * 4 * N])

                S5 = S_t[:].rearrange("p (n c two) -> p n c two", two=2, c=5)
                O5 = O_t[:].rearrange("p (n c two) -> p n c two", two=2, c=5)
                Kt22 = K_t[:].rearrange("p (n c two) -> p n c two", two=2, c=2)
                Kt4 = K_t[:].rearrange("p (n f) -> p n f", f=4)

                w3 = S5[:, :, 0:1, 0]     # [P,N,1]
                v3 = S5[:, :, 0:1, 1]
                wv3 = S5[:, :, 0, :]      # [P,N,2]
                a4 = S5[:, :, 1:5, 0]     # [P,N,4]
                b4 = S5[:, :, 1:5, 1]
                k02 = Kt22[:, :, :, 0]    # (k0,k2)
                k13 = Kt22[:, :, :, 1]    # (k1,k3)
                k01 = Kt4[:, :, 0:2]
                k23 = Kt4[:, :, 2:4]

                X_t = tp.tile([P, 2 * N], F32, tag="X")
                Y_t = tp.tile([P, 2 * N], F32, tag="Y")
                MD_t = tp2.tile([P, 2 * N], F32, tag="MD")
                Mb_t = tp.tile([P, N], F32, tag="Mb")
                ln_t = tp.tile([P, N], F32, tag="ln")
                A_t = tp2.tile([P, N], F32, tag="A")
                E_t = tp2.tile([P, N], F32, tag="E")
                PU_t = tp2.tile([P, 2 * N], F32, tag="PU")
                T3_t = tp.tile([P, N], F32, tag="T3")
                h_t = tp.tile([P, N], F32, tag="h")
                AE_t = tp.tile([P, N], F32, tag="AE")
                CMU_t = tp2.tile([P, 2 * N], F32, tag="CMU")
                ca0_t = tp.tile([P, N], F32, tag="ca0")
                CAB_t = tp2.tile([P, 2 * N], F32, tag="CAB")
                T4_t = tp.tile([P, 2 * N], F32, tag="T4")
                T5_t = tp.tile([P, 2 * N], F32, tag="T5")
                T6_t = tp.tile([P, 2 * N], F32, tag="T6")
                AB_t = tp2.tile([P, 2 * N], F32, tag="AB")
                T7a_t = tp.tile([P, 4 * N], F32, tag="T7a")
                T7b_t = tp.tile([P, 4 * N], F32, tag="T7b")
                T8_t = tp.tile([P, 4 * N], F32, tag="T8")
                G_t = tp.tile([P, 4 * N], F32, tag="G")

                X2 = X_t[:].rearrange("p (n two) -> p n two", two=2)
                Y2 = Y_t[:].rearrange("p (n two) -> p n two", two=2)
                MD2 = MD_t[:].rearrange("p (n two) -> p n two", two=2)
                PU2 = PU_t[:].rearrange("p (n two) -> p n two", two=2)
                CMU2 = CMU_t[:].rearrange("p (n two) -> p n two", two=2)
                CAB2 = CAB_t[:].rearrange("p (n two) -> p n two", two=2)
                AB2 = AB_t[:].rearrange("p (n two) -> p n two", two=2)
                T7a2 = T7a_t[:].rearrange("p (n f) -> p n f", f=4)
                T7b2 = T7b_t[:].rearrange("p (n f) -> p n f", f=4)
                T82 = T8_t[:].rearrange("p (n f) -> p n f", f=4)
                G2 = G_t[:].rearrange("p (n f) -> p n f", f=4)

                A3 = A_t[:].unsqueeze(2)
                E3 = E_t[:].unsqueeze(2)

                # X=(k0,k2)*w ; Y=(k1,k3)*v ; MD=X+Y=(mM,mD)
                nc.vector.tensor_mul(X2, k02, w3.broadcast_to([P, N, 2]))
                nc.vector.tensor_mul(Y2, k13, v3.broadcast_to([P, N, 2]))
                geng.tensor_add(MD_t[:], X_t[:], Y_t[:])

                # Mb = 10 - mM ; E = mD - 1 ; A = 1/Mb
                nc.scalar.activation(Mb_t[:].unsqueeze(2), MD2[:, :, 0:1], Copy,
                                     bias=10.0, scale=-1.0)
                nc.scalar.activation(E3, MD2[:, :, 1:2], Copy,
                                     bias=-1.0, scale=1.0)
                # A = 1/Mb via exp(-ln(Mb)) on ACT (Mb > 0 always: Mb = 10 - mM)
                nc.scalar.activation(ln_t[:], Mb_t[:],
                                     mybir.ActivationFunctionType.Ln)
                nc.scalar.activation(A_t[:], ln_t[:],
                                     mybir.ActivationFunctionType.Exp, scale=-1.0)

                # u = 0.2w + v ; T3 = E*u ; h = -4w + T3 ; P = (h+0.02)*A
                nc.vector.scalar_tensor_tensor(PU2[:, :, 0:1], w3, 0.2, v3, mul, add)
                nc.vector.tensor_mul(T3_t[:].unsqueeze(2), E3, PU2[:, :, 0:1])
                nc.vector.scalar_tensor_tensor(h_t[:].unsqueeze(2), w3, -4.0,
                                               T3_t[:].unsqueeze(2), mul, add)
                nc.vector.scalar_tensor_tensor(PU2[:, :, 1:2], h_t[:].unsqueeze(2),
                                               0.02, A3, add, mul)

                # AE = A*E ; (c_u,c_m) = A*(u,P)
                nc.vector.tensor_mul(AE_t[:].unsqueeze(2), A3, E3)
                nc.vector.tensor_mul(CMU2, A3.broadcast_to([P, N, 2]), PU2)

                # c_a = 0.2AE - 4A ; c_b = AE - 0.2
                nc.scalar.activation(ca0_t[:].unsqueeze(2), A3, Copy, scale=-4.0)
                nc.vector.scalar_tensor_tensor(CAB2[:, :, 0:1], AE_t[:].unsqueeze(2),
                                               0.2, ca0_t[:].unsqueeze(2), mul, add)
                nc.scalar.activation(CAB2[:, :, 1:2], AE_t[:].unsqueeze(2), Copy,
                                     bias=-0.2, scale=1.0)

                c_u_bc2 = CMU2[:, :, 0:1].broadcast_to([P, N, 2])
                c_m_bc2 = CMU2[:, :, 1:2].broadcast_to([P, N, 2])

                # (alpha,beta) = (c_a,c_b) + c_u*(k2,k3) + c_m*(k0,k1)
                nc.vector.tensor_mul(
                    T4_t[:].rearrange("p (n two) -> p n two", two=2), c_u_bc2, k23)
                nc.vector.tensor_mul(
                    T5_t[:].rearrange("p (n two) -> p n two", two=2), c_m_bc2, k01)
                geng.tensor_add(T6_t[:], T4_t[:], T5_t[:])
                geng.tensor_add(AB_t[:], T6_t[:], CAB_t[:])

                # f2 = P - 0.2v -> out col 1
                nc.vector.scalar_tensor_tensor(O5[:, :, 0:1, 1], v3, -0.2,
                                               PU2[:, :, 1:2], mul, add)

                # df2_p = alpha*a_p + beta*b_p + gamma_p -> out cols 3,5,7,9
                nc.vector.tensor_mul(T7a2, AB2[:, :, 0:1].broadcast_to([P, N, 4]), a4)
                nc.vector.tensor_mul(T7b2, AB2[:, :, 1:2].broadcast_to([P, N, 4]), b4)
                geng.tensor_add(T8_t[:], T7a_t[:], T7b_t[:])
                nc.vector.tensor_mul(G2[:, :, 0:2], c_m_bc2, wv3)
                nc.vector.tensor_mul(G2[:, :, 2:4], c_u_bc2, wv3)
                geng.tensor_add(O5[:, :, 1:5, 1], T82, G2)

                # out even cols = state odd cols
                nc.scalar.activation(O5[:, :, :, 0], S5[:, :, :, 1], Copy)

                nc.sync.dma_start(ov[:, c * 10 * N:(c + 1) * 10 * N], O_t[:])

    orig = nc.to_json_bytes
    nc.to_json_bytes = lambda: _fix_bir_json(orig())
    return nc


def _build2(R: int, N: int, reps: int = 1, chunks: int = 7):
    """v2: single-engine (DVE-only) minimal-instruction design.

    This platform charges a large fixed cost per engine instruction, so the
    kernel is organised as ~18 wide DVE ops per chunk, no cross-engine sync
    (outputs are computed in-place in the input state tile), HWDGE DMAs.
    """
    import concourse.bass as bass
    import concourse.tile as tile
    import concourse.mybir as mybir
    from concourse.ap import AP

    F32 = mybir.dt.float32
    mul = mybir.AluOpType.mult
    add = mybir.AluOpType.add
    sub = mybir.AluOpType.subtract

    nc = bass.Bass("TRN2")
    state_d = nc.dram_tensor("state", [R, 10], F32, kind="ExternalInput")
    k_d = nc.dram_tensor("K", [R, 4], F32, kind="ExternalInput")
    out_d = nc.dram_tensor("out", [R, 10], F32, kind="ExternalOutput")
    sv = state_d[:].rearrange("(p n) m -> p (n m)", p=P)
    kv = k_d[:].rearrange("(p n) m -> p (n m)", p=P)
    ov = out_d[:].rearrange("(p n) m -> p (n m)", p=P)

    def mkap(tile_ap, offset, dims):
        # dims: list of [step, count] free dims; partition dim taken from tile
        part = tile_ap.ap[0]
        return AP(tile_ap.tensor, offset, [list(part)] + [list(d) for d in dims])

    with tile.TileContext(nc) as tc:
        with (
            tc.tile_pool(name="io", bufs=2) as io,
            tc.tile_pool(name="tmp", bufs=1) as tp,
            tc.tile_pool(name="const", bufs=1) as cp,
        ):
            C2 = cp.tile([P, 2], F32)      # [10, 1]
            ones = cp.tile([P, 1], F32)
            nc.vector.memset(C2[:, 0:1], 10.0)
            nc.vector.memset(C2[:, 1:2], 1.0)
            nc.vector.memset(ones[:], 1.0)

            for c in [c for _ in range(reps) for c in range(chunks)]:
                S_t = io.tile([P, 10 * N], F32, tag="S")
                K_t = io.tile([P, 4 * N], F32, tag="K")
                nc.sync.dma_start(S_t[:], sv[:, c * 10 * N:(c + 1) * 10 * N])
                nc.sync.dma_start(K_t[:], kv[:, c * 4 * N:(c + 1) * 4 * N])

                SC = tp.tile([P, 20 * N], F32, tag="SC")
                ZZ = tp.tile([P, 10 * N], F32, tag="ZZ")
                U5_t = tp.tile([P, 5 * N], F32, tag="U5")
                DU5_t = tp.tile([P, 5 * N], F32, tag="DU5")
                H5_t = tp.tile([P, 5 * N], F32, tag="H5")
                MD_t = tp.tile([P, 2 * N], F32, tag="MD")
                A_t = tp.tile([P, N], F32, tag="A")
                P_t = tp.tile([P, N], F32, tag="P")
                cm_t = tp.tile([P, N], F32, tag="cm")

                S5 = S_t[:].rearrange("p (n c two) -> p n c two", two=2, c=5)
                evens = S5[:, :, :, 0]                    # [P,N,5] strides (10,2)
                odds = S5[:, :, :, 1]
                ev_rep = evens.unsqueeze(2).broadcast_to([P, N, 2, 5])
                od_rep = odds.unsqueeze(2).broadcast_to([P, N, 2, 5])
                Kt22 = K_t[:].rearrange("p (n c two) -> p n c two", two=2, c=2)
                K02 = Kt22[:, :, :, 0].unsqueeze(3).broadcast_to([P, N, 2, 5])
                K13 = Kt22[:, :, :, 1].unsqueeze(3).broadcast_to([P, N, 2, 5])

                E2v = SC[:, :10 * N].rearrange("p (n a c) -> p n a c", a=2, c=5)
                Rv = SC[:, 10 * N:].rearrange("p (n a c) -> p n a c", a=2, c=5)
                ZZv = ZZ[:].rearrange("p (n a c) -> p n a c", a=2, c=5)
                U5v = U5_t[:].rearrange("p (n c) -> p n c", c=5)
                DU5v = DU5_t[:].rearrange("p (n c) -> p n c", c=5)
                H5v = H5_t[:].rearrange("p (n c) -> p n c", c=5)
                MDv = MD_t[:].rearrange("p (n c) -> p n c", c=2)
                A3 = A_t[:].unsqueeze(2)                  # [P,N,1]
                P3 = P_t[:].unsqueeze(2)
                cm3 = cm_t[:].unsqueeze(2)

                # 1-3: ZZ[j2,c] = k_{2j2}*S[2c] + k_{2j2+1}*S[2c+1]
                nc.vector.tensor_mul(E2v, K02, ev_rep)
                nc.vector.tensor_mul(Rv, K13, od_rep)
                nc.vector.tensor_add(ZZv, E2v, Rv)
                # 4: extras — ZZ slots {1,2,8,9} += (w,v,w,v)
                zz_ex = mkap(ZZ[:], 1, [[10, N], [7, 2], [1, 2]])
                wv_rep = mkap(S_t[:], 0, [[10, N], [0, 2], [1, 2]])
                nc.vector.tensor_add(zz_ex, zz_ex, wv_rep)
                # 5: MD = [10,1] - [mM, mD]
                c2b = mkap(C2[:], 0, [[0, N], [1, 2]])
                zz0 = mkap(ZZ[:], 0, [[10, N], [5, 2]])
                nc.vector.tensor_tensor(MDv, c2b, zz0, sub)
                # 6: A = 1/M
                nc.vector.reciprocal(A_t[:], MDv[:, :, 0])
                # 7: U5 = 0.2*evens + odds
                nc.vector.scalar_tensor_tensor(U5v, evens, 0.2, odds, mul, add)
                # 8: DU5 = D * U5
                nc.vector.tensor_mul(DU5v, MDv[:, :, 1:2].broadcast_to([P, N, 5]), U5v)
                # 9: NDU = u * nD_p   (SC[0:4N])
                NDU = SC[:, :4 * N].rearrange("p (n c) -> p n c", c=4)
                nc.vector.tensor_mul(NDU, U5v[:, :, 0:1].broadcast_to([P, N, 4]),
                                     ZZv[:, :, 1, 1:5])
                # 10: H5 = -4*evens - DU5
                nc.vector.scalar_tensor_tensor(H5v, evens, -4.0, DU5v, mul, sub)
                # 11: DG4 = H5[1:5] + NDU   (SC[4N:8N])
                DG4 = SC[:, 4 * N:8 * N].rearrange("p (n c) -> p n c", c=4)
                nc.vector.tensor_add(DG4, H5v[:, :, 1:5], NDU)
                # 12: P = (H5[0] + 0.02) * A
                nc.vector.scalar_tensor_tensor(P3, H5v[:, :, 0:1], 0.02, A3, add, mul)
                # 13: cm = A * P
                nc.vector.tensor_mul(cm3, A3, P3)
                # 14: Q4 = A * DG4   (SC[8N:12N])
                Q4 = SC[:, 8 * N:12 * N].rearrange("p (n c) -> p n c", c=4)
                nc.vector.tensor_mul(Q4, A3.broadcast_to([P, N, 4]), DG4)
                # 15: R4 = cm * nM_p   (SC[12N:16N])
                R4 = SC[:, 12 * N:16 * N].rearrange("p (n c) -> p n c", c=4)
                nc.vector.tensor_mul(R4, cm3.broadcast_to([P, N, 4]),
                                     ZZv[:, :, 0, 1:5])
                # 16: S4 = Q4 + R4   (SC[16N:20N])
                S4 = SC[:, 16 * N:20 * N].rearrange("p (n c) -> p n c", c=4)
                nc.vector.tensor_add(S4, Q4, R4)
                # 17: shift evens <- odds (out even cols = state odd cols)
                nc.vector.tensor_mul(evens, odds,
                                     mkap(ones[:], 0, [[0, N], [0, 5]]))
                # 18: df2 slots (S odd cols 3,5,7,9) = -0.2*b4 + S4
                b4 = S5[:, :, 1:5, 1]
                nc.vector.scalar_tensor_tensor(b4, b4, -0.2, S4, mul, add)
                # 19: f2 (S col 1) = -0.2*v + P
                v3 = S5[:, :, 0:1, 1]
                nc.vector.scalar_tensor_tensor(v3, v3, -0.2, P3, mul, add)

                nc.sync.dma_start(ov[:, c * 10 * N:(c + 1) * 10 * N], S_t[:])

    orig = nc.to_json_bytes
    nc.to_json_bytes = lambda: _fix_bir_json(orig())
    return nc


def _build3(R: int, N: int, reps: int = 1, chunks: int = 6,
            staggered: bool = False):
    """v3: v2's math inside a hardware For_i loop over chunks.

    On this platform, first-time instruction streaming costs ~20-100us per
    instruction, but loop iterations re-execute from IRAM at normal speed —
    so the chunk pipeline is emitted once and looped with dynamic DMA
    offsets."""
    import concourse.bass as bass
    import concourse.tile as tile
    import concourse.mybir as mybir
    from concourse.ap import AP

    F32 = mybir.dt.float32
    mul = mybir.AluOpType.mult
    add = mybir.AluOpType.add
    sub = mybir.AluOpType.subtract

    nc = bass.Bass("TRN2")
    state_d = nc.dram_tensor("state", [R, 10], F32, kind="ExternalInput")
    k_d = nc.dram_tensor("K", [R, 4], F32, kind="ExternalInput")
    out_d = nc.dram_tensor("out", [R, 10], F32, kind="ExternalOutput")
    sv = state_d[:].rearrange("(p n) m -> p (n m)", p=P)
    kv = k_d[:].rearrange("(p n) m -> p (n m)", p=P)
    ov = out_d[:].rearrange("(p n) m -> p (n m)", p=P)

    def mkap(tile_ap, offset, dims):
        part = tile_ap.ap[0]
        return AP(tile_ap.tensor, offset, [list(part)] + [list(d) for d in dims])

    with tile.TileContext(nc) as tc:
        with (
            tc.tile_pool(name="io", bufs=1) as io,
            tc.tile_pool(name="tmp", bufs=1) as tp,
            tc.tile_pool(name="const", bufs=1) as cp,
        ):
            C2 = cp.tile([P, 2], F32)
            ones = cp.tile([P, 1], F32)
            nc.vector.memset(C2[:, 0:1], 10.0)
            nc.vector.memset(C2[:, 1:2], 1.0)
            nc.vector.memset(ones[:], 1.0)

            with tc.For_i(0, chunks * reps, 1, staggered_reset=staggered) as iv:
                off = iv if reps == 1 else iv * 0

                S_t = io.tile([P, 10 * N], F32, tag="S")
                K_t = io.tile([P, 4 * N], F32, tag="K")
                nc.sync.dma_start(S_t[:], sv[:, bass.ts(off, 10 * N)])
                nc.sync.dma_start(K_t[:], kv[:, bass.ts(off, 4 * N)])

                SC = tp.tile([P, 20 * N], F32, tag="SC")
                ZZ = tp.tile([P, 10 * N], F32, tag="ZZ")
                U5_t = tp.tile([P, 5 * N], F32, tag="U5")
                DU5_t = tp.tile([P, 5 * N], F32, tag="DU5")
                H5_t = tp.tile([P, 5 * N], F32, tag="H5")
                MD_t = tp.tile([P, 2 * N], F32, tag="MD")
                A_t = tp.tile([P, N], F32, tag="A")
                P_t = tp.tile([P, N], F32, tag="P")
                cm_t = tp.tile([P, N], F32, tag="cm")

                S5 = S_t[:].rearrange("p (n c two) -> p n c two", two=2, c=5)
                evens = S5[:, :, :, 0]
                odds = S5[:, :, :, 1]
                ev_rep = evens.unsqueeze(2).broadcast_to([P, N, 2, 5])
                od_rep = odds.unsqueeze(2).broadcast_to([P, N, 2, 5])
                Kt22 = K_t[:].rearrange("p (n c two) -> p n c two", two=2, c=2)
                K02 = Kt22[:, :, :, 0].unsqueeze(3).broadcast_to([P, N, 2, 5])
                K13 = Kt22[:, :, :, 1].unsqueeze(3).broadcast_to([P, N, 2, 5])

                E2v = SC[:, :10 * N].rearrange("p (n a c) -> p n a c", a=2, c=5)
                Rv = SC[:, 10 * N:].rearrange("p (n a c) -> p n a c", a=2, c=5)
                ZZv = ZZ[:].rearrange("p (n a c) -> p n a c", a=2, c=5)
                U5v = U5_t[:].rearrange("p (n c) -> p n c", c=5)
                DU5v = DU5_t[:].rearrange("p (n c) -> p n c", c=5)
                H5v = H5_t[:].rearrange("p (n c) -> p n c", c=5)
                MDv = MD_t[:].rearrange("p (n c) -> p n c", c=2)
                A3 = A_t[:].unsqueeze(2)
                P3 = P_t[:].unsqueeze(2)
                cm3 = cm_t[:].unsqueeze(2)

                nc.vector.tensor_mul(E2v, K02, ev_rep)
                nc.vector.tensor_mul(Rv, K13, od_rep)
                nc.vector.tensor_add(ZZv, E2v, Rv)
                zz_ex = mkap(ZZ[:], 1, [[10, N], [7, 2], [1, 2]])
                wv_rep = mkap(S_t[:], 0, [[10, N], [0, 2], [1, 2]])
                nc.vector.tensor_add(zz_ex, zz_ex, wv_rep)
                c2b = mkap(C2[:], 0, [[0, N], [1, 2]])
                zz0 = mkap(ZZ[:], 0, [[10, N], [5, 2]])
                nc.vector.tensor_tensor(MDv, c2b, zz0, sub)
                nc.vector.reciprocal(A_t[:], MDv[:, :, 0])
                nc.vector.scalar_tensor_tensor(U5v, evens, 0.2, odds, mul, add)
                nc.vector.tensor_mul(DU5v, MDv[:, :, 1:2].broadcast_to([P, N, 5]),
                                     U5v)
                NDU = SC[:, :4 * N].rearrange("p (n c) -> p n c", c=4)
                nc.vector.tensor_mul(NDU, U5v[:, :, 0:1].broadcast_to([P, N, 4]),
                                     ZZv[:, :, 1, 1:5])
                nc.vector.scalar_tensor_tensor(H5v, evens, -4.0, DU5v, mul, sub)
                DG4 = SC[:, 4 * N:8 * N].rearrange("p (n c) -> p n c", c=4)
                nc.vector.tensor_add(DG4, H5v[:, :, 1:5], NDU)
                nc.vector.scalar_tensor_tensor(P3, H5v[:, :, 0:1], 0.02, A3,
                                               add, mul)
                nc.vector.tensor_mul(cm3, A3, P3)
                Q4 = SC[:, 8 * N:12 * N].rearrange("p (n c) -> p n c", c=4)
                nc.vector.tensor_mul(Q4, A3.broadcast_to([P, N, 4]), DG4)
                R4 = SC[:, 12 * N:16 * N].rearrange("p (n c) -> p n c", c=4)
                nc.vector.tensor_mul(R4, cm3.broadcast_to([P, N, 4]),
                                     ZZv[:, :, 0, 1:5])
                S4 = SC[:, 16 * N:20 * N].rearrange("p (n c) -> p n c", c=4)
                nc.vector.tensor_add(S4, Q4, R4)
                nc.vector.tensor_mul(evens, odds,
                                     mkap(ones[:], 0, [[0, N], [0, 5]]))
                b4 = S5[:, :, 1:5, 1]
                nc.vector.scalar_tensor_tensor(b4, b4, -0.2, S4, mul, add)
                v3 = S5[:, :, 0:1, 1]
                nc.vector.scalar_tensor_tensor(v3, v3, -0.2, P3, mul, add)

                nc.sync.dma_start(ov[:, bass.ts(off, 10 * N)], S_t[:])

    orig = nc.to_json_bytes
    nc.to_json_bytes = lambda: _fix_bir_json(orig())
    return nc


def _build4(R: int, N: int, reps: int = 1, chunks: int = 8, parts: str = "all"):
    """v4: alpha/beta factorization (~56 elem/row vs v3's 84) split across
    DVE (~33N) / GPSIMD (~14N) / ACT (~9N), with DMA double-buffered by
    tracing TWO chunk pipelines per For_i iteration on bufs=2 pools.

    Math per row (w=omega, v=omega_dot, K=(k0..k3), aug pairs (a_p, b_p)):
        mM = k0 w + k1 v ; mD = k2 w + k3 v ; M = 10-mM ; D = 1-mD ; A = 1/M
        u = 0.2w + v ; h = -4w - D*u ; P = (h+0.02)A ; f2 = P - 0.2v
        cu = Au ; cm = AP ; AD = AD
        alpha = -4A - 0.2AD + cu k2 + cm k0 ; betaX = -AD + cu k3 + cm k1
        df_p = alpha a_p + betaX b_p + gamma_p - 0.2 b_p
        gamma = (cm w, cm v, cu w, cu v)
    """
    import concourse.bass as bass
    import concourse.tile as tile
    import concourse.mybir as mybir

    F32 = mybir.dt.float32
    mul = mybir.AluOpType.mult
    add = mybir.AluOpType.add
    sub = mybir.AluOpType.subtract
    Copy = mybir.ActivationFunctionType.Copy
    Ln = mybir.ActivationFunctionType.Ln
    Exp = mybir.ActivationFunctionType.Exp

    assert chunks % 2 == 0
    nc = bass.Bass("TRN2")
    geng = nc.gpsimd
    state_d = nc.dram_tensor("state", [R, 10], F32, kind="ExternalInput")
    k_d = nc.dram_tensor("K", [R, 4], F32, kind="ExternalInput")
    out_d = nc.dram_tensor("out", [R, 10], F32, kind="ExternalOutput")
    sv = state_d[:].rearrange("(p n) m -> p (n m)", p=P)
    kv = k_d[:].rearrange("(p n) m -> p (n m)", p=P)
    ov = out_d[:].rearrange("(p n) m -> p (n m)", p=P)

    with tile.TileContext(nc) as tc:
        with (
            tc.tile_pool(name="io", bufs=2) as io,
            tc.tile_pool(name="tmp", bufs=2) as tp,
            tc.tile_pool(name="pre", bufs=1) as pre,
        ):
            # Preload the ln/exp ACT table outside the loop so walrus doesn't
            # place a table-load inside the hot loop body.
            warm = pre.tile([P, 1], F32)
            b10 = pre.tile([P, 1], F32)
            nc.vector.memset(warm[:], 1.0)
            nc.vector.memset(b10[:], 10.0)
            nc.scalar.activation(warm[:], warm[:], Ln, bias=b10[:])
            nc.scalar.activation(warm[:], warm[:], Exp)

            dve = parts in ("all", "dve", "dve+gp", "dve+act")
            gp = parts in ("all", "gp", "dve+gp")
            act = parts in ("all", "act", "dve+act")

            def tick(c_expr):
                S_t = io.tile([P, 10 * N], F32, tag="S")
                K_t = io.tile([P, 4 * N], F32, tag="K")
                nc.sync.dma_start(S_t[:], sv[:, bass.ts(c_expr, 10 * N)])
                nc.sync.dma_start(K_t[:], kv[:, bass.ts(c_expr, 4 * N)])

                S5 = S_t[:].rearrange("p (n c two) -> p n c two", two=2, c=5)
                w3 = S5[:, :, 0:1, 0]          # [P,N,1]
                v3 = S5[:, :, 0:1, 1]
                wv = S5[:, :, 0, :]            # [P,N,2] contiguous (w,v)
                a4 = S5[:, :, 1:5, 0]
                b4 = S5[:, :, 1:5, 1]
                evens = S5[:, :, :, 0]
                odds = S5[:, :, :, 1]
                Kt22 = K_t[:].rearrange("p (n c two) -> p n c two", two=2, c=2)
                Kt4 = K_t[:].rearrange("p (n f) -> p n f", f=4)
                k02 = Kt22[:, :, :, 0]         # (k0,k2)
                k13 = Kt22[:, :, :, 1]         # (k1,k3)
                k01 = Kt4[:, :, 0:2]
                k23 = Kt4[:, :, 2:4]

                X2_t = tp.tile([P, 2 * N], F32, tag="X2")   # X2, then Z2
                Y2_t = tp.tile([P, 2 * N], F32, tag="Y2")
                u_t = tp.tile([P, N], F32, tag="u")
                t_t = tp.tile([P, N], F32, tag="t")         # t, then h, then P
                cu_t = tp.tile([P, N], F32, tag="cu")
                cm_t = tp.tile([P, N], F32, tag="cm")
                AD_t = tp.tile([P, N], F32, tag="AD")       # AD, then betaX
                t12_t = tp.tile([P, 2 * N], F32, tag="t12")  # t12, then t36
                t45_t = tp.tile([P, 2 * N], F32, tag="t45")
                s_t = tp.tile([P, N], F32, tag="s")         # s, then alpha
                Ta_t = tp.tile([P, 4 * N], F32, tag="Ta")   # Ta4, then T8, S4
                Tb_t = tp.tile([P, 4 * N], F32, tag="Tb")
                G_t = tp.tile([P, 4 * N], F32, tag="G")
                A_t = tp.tile([P, N], F32, tag="A")         # lnM, then A
                A4_t = tp.tile([P, N], F32, tag="A4")
                D_t = tp.tile([P, N], F32, tag="D")

                X2 = X2_t[:].rearrange("p (n c) -> p n c", c=2)
                Y2 = Y2_t[:].rearrange("p (n c) -> p n c", c=2)
                t12 = t12_t[:].rearrange("p (n c) -> p n c", c=2)
                t45 = t45_t[:].rearrange("p (n c) -> p n c", c=2)
                Ta = Ta_t[:].rearrange("p (n c) -> p n c", c=4)
                Tb = Tb_t[:].rearrange("p (n c) -> p n c", c=4)
                G = G_t[:].rearrange("p (n c) -> p n c", c=4)
                u3 = u_t[:].unsqueeze(2)
                t3 = t_t[:].unsqueeze(2)
                cu3 = cu_t[:].unsqueeze(2)
                cm3 = cm_t[:].unsqueeze(2)
                AD3 = AD_t[:].unsqueeze(2)
                s3 = s_t[:].unsqueeze(2)
                A3 = A_t[:].unsqueeze(2)
                A43 = A4_t[:].unsqueeze(2)
                D3 = D_t[:].unsqueeze(2)

                # Timing-probe fallbacks: when a producer engine is disabled,
                # its consumers read an input-backed view of the same shape.
                A3r = A3 if act else u3 if dve else w3
                D3r = D3 if act else u3 if dve else w3
                A43r = A43 if act else u3 if dve else w3
                X2r = X2_t[:] if dve else S_t[:, :2 * N]
                Y2r = Y2_t[:] if dve else S_t[:, 2 * N:4 * N]
                X2v = X2 if (dve or gp) else S5[:, :, 1:3, 0]
                t12r = t12_t[:] if dve else S_t[:, :2 * N]
                t45r = t45_t[:] if dve else S_t[:, 2 * N:4 * N]
                s3r = s3 if dve else w3
                AD3r = AD3 if dve else v3
                Tar = Ta_t[:] if dve else S_t[:, :4 * N]
                Tbr = Tb_t[:] if dve else S_t[:, 4 * N:8 * N]
                Gr = G_t[:] if dve else S_t[:, 4 * N:8 * N]
                Tav = Ta if dve else G
                s3g = s3 if (dve or gp) else w3
                AD3g = AD3 if (dve or gp) else v3

                # --- front: mM/mD and the per-row scalars ---
                if dve:
                    nc.vector.tensor_mul(X2, k02, w3.broadcast_to([P, N, 2]))
                    nc.vector.tensor_mul(Y2, k13, v3.broadcast_to([P, N, 2]))
                    nc.vector.scalar_tensor_tensor(u3, w3, 0.2, v3, mul, add)
                if gp:
                    geng.tensor_add(X2_t[:], X2r, Y2r)  # Z2=(mM,mD)
                if act:
                    # lnM = Ln(10 - mM) ; A = Exp(-lnM) ; D = 1-mD ; A4 = -4A
                    nc.scalar.activation(A3, X2v[:, :, 0:1], Ln, bias=b10[:],
                                         scale=-1.0)
                    nc.scalar.activation(A3, A3, Exp, scale=-1.0)
                    nc.scalar.activation(D3, X2v[:, :, 1:2], Copy, bias=1.0,
                                         scale=-1.0)
                    nc.scalar.activation(A43, A3, Copy, scale=-4.0)

                if dve:
                    nc.vector.tensor_mul(t3, D3r, u3)                 # t = D*u
                    nc.vector.scalar_tensor_tensor(t3, w3, -4.0, t3, mul, sub)
                    nc.vector.scalar_tensor_tensor(t3, t3, 0.02, A3r, add, mul)
                    nc.vector.tensor_mul(cu3, A3r, u3)
                    nc.vector.tensor_mul(cm3, A3r, t3)
                    nc.vector.tensor_mul(AD3, A3r, D3r)
                    nc.vector.tensor_mul(t12, cu3.broadcast_to([P, N, 2]), k23)
                    nc.vector.tensor_mul(t45, cm3.broadcast_to([P, N, 2]), k01)
                    nc.vector.scalar_tensor_tensor(s3, AD3, -0.2, A43r, mul, add)
                if gp:
                    geng.tensor_add(t12_t[:], t12r, t45r)          # t36
                    geng.tensor_add(s3, t12[:, :, 0:1], s3r)       # alpha
                    geng.tensor_sub(AD3, t12[:, :, 1:2], AD3r)     # betaX

                # --- per-p tail ---
                if dve:
                    nc.vector.tensor_mul(Ta, s3g.broadcast_to([P, N, 4]), a4)
                    nc.vector.tensor_mul(Tb, AD3g.broadcast_to([P, N, 4]), b4)
                    nc.vector.tensor_mul(G[:, :, 0:2],
                                         cm3.broadcast_to([P, N, 2]), wv)
                    nc.vector.tensor_mul(G[:, :, 2:4],
                                         cu3.broadcast_to([P, N, 2]), wv)
                if act:
                    # out even cols = state odd cols (reads odds BEFORE writes)
                    nc.scalar.activation(evens, odds, Copy)
                if gp:
                    geng.tensor_add(Ta_t[:], Tar, Tbr)             # T8
                    geng.tensor_add(Ta_t[:], Ta_t[:], Gr)          # S4
                if dve:
                    nc.vector.scalar_tensor_tensor(S5[:, :, 0:1, 1], v3, -0.2,
                                                   t3, mul, add)  # f2 -> out[1]
                    nc.vector.scalar_tensor_tensor(b4, b4, -0.2, Tav, mul, add)

                nc.sync.dma_start(ov[:, bass.ts(c_expr, 10 * N)], S_t[:])

            with tc.For_i(0, (chunks // 2) * reps, 1) as iv:
                base = iv if reps == 1 else iv * 0
                tick(base * 2)
                tick(base * 2 + 1)

    orig = nc.to_json_bytes
    nc.to_json_bytes = lambda: _fix_bir_json(orig())
    return nc


def _build5(R: int, N: int, reps: int = 1, chunks: int = 10,
            staggered: bool = True):
    """v5: v4's math with the two ticks per For_i iteration interleaved
    phase-by-phase, so each engine streams tick B work while the others
    chew tick A — in-order engine queues stop serializing on the
    cross-engine dependency chain. Loads ride the SP HWDGE ring, stores
    the ACT ring (independent FIFOs); io is triple-buffered so the next
    iteration's loads never wait on the current compute."""
    import concourse.bass as bass
    import concourse.tile as tile
    import concourse.mybir as mybir

    F32 = mybir.dt.float32
    mul = mybir.AluOpType.mult
    add = mybir.AluOpType.add
    sub = mybir.AluOpType.subtract
    Copy = mybir.ActivationFunctionType.Copy
    Ln = mybir.ActivationFunctionType.Ln
    Exp = mybir.ActivationFunctionType.Exp

    assert chunks % 2 == 0
    nc = bass.Bass("TRN2")
    geng = nc.gpsimd
    state_d = nc.dram_tensor("state", [R, 10], F32, kind="ExternalInput")
    k_d = nc.dram_tensor("K", [R, 4], F32, kind="ExternalInput")
    out_d = nc.dram_tensor("out", [R, 10], F32, kind="ExternalOutput")
    sv = state_d[:].rearrange("(p n) m -> p (n m)", p=P)
    kv = k_d[:].rearrange("(p n) m -> p (n m)", p=P)
    ov = out_d[:].rearrange("(p n) m -> p (n m)", p=P)

    with tile.TileContext(nc) as tc:
        with (
            tc.tile_pool(name="io", bufs=3) as io,
            tc.tile_pool(name="tmp", bufs=2) as tp,
            tc.tile_pool(name="pre", bufs=1) as pre,
        ):
            warm = pre.tile([P, 1], F32)
            b10 = pre.tile([P, 1], F32)
            nc.vector.memset(warm[:], 1.0)
            nc.vector.memset(b10[:], 10.0)
            nc.scalar.activation(warm[:], warm[:], Ln, bias=b10[:])
            nc.scalar.activation(warm[:], warm[:], Exp)

            class Tick:
                pass

            def alloc(c_expr):
                t = Tick()
                t.c = c_expr
                t.S_t = io.tile([P, 10 * N], F32, tag="S")
                t.K_t = io.tile([P, 4 * N], F32, tag="K")
                nc.sync.dma_start(t.S_t[:], sv[:, bass.ts(c_expr, 10 * N)])
                nc.sync.dma_start(t.K_t[:], kv[:, bass.ts(c_expr, 4 * N)])

                S5 = t.S_t[:].rearrange("p (n c two) -> p n c two", two=2, c=5)
                t.S5 = S5
                t.w3 = S5[:, :, 0:1, 0]
                t.v3 = S5[:, :, 0:1, 1]
                t.wv = S5[:, :, 0, :]
                t.a4 = S5[:, :, 1:5, 0]
                t.b4 = S5[:, :, 1:5, 1]
                t.evens = S5[:, :, :, 0]
                t.odds = S5[:, :, :, 1]
                Kt22 = t.K_t[:].rearrange("p (n c two) -> p n c two", two=2, c=2)
                Kt4 = t.K_t[:].rearrange("p (n f) -> p n f", f=4)
                t.k02 = Kt22[:, :, :, 0]
                t.k13 = Kt22[:, :, :, 1]
                t.k01 = Kt4[:, :, 0:2]
                t.k23 = Kt4[:, :, 2:4]

                t.X2_t = tp.tile([P, 2 * N], F32, tag="X2")
                t.Y2_t = tp.tile([P, 2 * N], F32, tag="Y2")
                t.u_t = tp.tile([P, N], F32, tag="u")
                t.t_t = tp.tile([P, N], F32, tag="t")
                t.cu_t = tp.tile([P, N], F32, tag="cu")
                t.cm_t = tp.tile([P, N], F32, tag="cm")
                t.AD_t = tp.tile([P, N], F32, tag="AD")
                t.t12_t = tp.tile([P, 2 * N], F32, tag="t12")
                t.t45_t = tp.tile([P, 2 * N], F32, tag="t45")
                t.s_t = tp.tile([P, N], F32, tag="s")
                t.Ta_t = tp.tile([P, 4 * N], F32, tag="Ta")
                t.Tb_t = tp.tile([P, 4 * N], F32, tag="Tb")
                t.G_t = tp.tile([P, 4 * N], F32, tag="G")
                t.A_t = tp.tile([P, N], F32, tag="A")
                t.A4_t = tp.tile([P, N], F32, tag="A4")
                t.D_t = tp.tile([P, N], F32, tag="D")

                t.X2 = t.X2_t[:].rearrange("p (n c) -> p n c", c=2)
                t.Y2 = t.Y2_t[:].rearrange("p (n c) -> p n c", c=2)
                t.t12 = t.t12_t[:].rearrange("p (n c) -> p n c", c=2)
                t.t45 = t.t45_t[:].rearrange("p (n c) -> p n c", c=2)
                t.Ta = t.Ta_t[:].rearrange("p (n c) -> p n c", c=4)
                t.Tb = t.Tb_t[:].rearrange("p (n c) -> p n c", c=4)
                t.G = t.G_t[:].rearrange("p (n c) -> p n c", c=4)
                t.u3 = t.u_t[:].unsqueeze(2)
                t.t3 = t.t_t[:].unsqueeze(2)
                t.cu3 = t.cu_t[:].unsqueeze(2)
                t.cm3 = t.cm_t[:].unsqueeze(2)
                t.AD3 = t.AD_t[:].unsqueeze(2)
                t.s3 = t.s_t[:].unsqueeze(2)
                t.A3 = t.A_t[:].unsqueeze(2)
                t.A43 = t.A4_t[:].unsqueeze(2)
                t.D3 = t.D_t[:].unsqueeze(2)
                return t

            def front(t):
                nc.vector.tensor_mul(t.X2, t.k02, t.w3.broadcast_to([P, N, 2]))
                nc.vector.tensor_mul(t.Y2, t.k13, t.v3.broadcast_to([P, N, 2]))
                nc.vector.scalar_tensor_tensor(t.u3, t.w3, 0.2, t.v3, mul, add)

            def gp_z2(t):
                geng.tensor_add(t.X2_t[:], t.X2_t[:], t.Y2_t[:])

            def acts(t):
                nc.scalar.activation(t.A3, t.X2[:, :, 0:1], Ln, bias=b10[:],
                                     scale=-1.0)
                nc.scalar.activation(t.A3, t.A3, Exp, scale=-1.0)
                nc.scalar.activation(t.D3, t.X2[:, :, 1:2], Copy, bias=1.0,
                                     scale=-1.0)
                nc.scalar.activation(t.A43, t.A3, Copy, scale=-4.0)

            def mid(t):
                nc.vector.tensor_mul(t.t3, t.D3, t.u3)
                nc.vector.scalar_tensor_tensor(t.t3, t.w3, -4.0, t.t3, mul, sub)
                nc.vector.scalar_tensor_tensor(t.t3, t.t3, 0.02, t.A3, add, mul)
                nc.vector.tensor_mul(t.cu3, t.A3, t.u3)
                nc.vector.tensor_mul(t.cm3, t.A3, t.t3)
                nc.vector.tensor_mul(t.AD3, t.A3, t.D3)
                nc.vector.tensor_mul(t.t12, t.cu3.broadcast_to([P, N, 2]), t.k23)
                nc.vector.tensor_mul(t.t45, t.cm3.broadcast_to([P, N, 2]), t.k01)
                nc.vector.scalar_tensor_tensor(t.s3, t.AD3, -0.2, t.A43, mul, add)

            def gp_mid(t):
                geng.tensor_add(t.t12_t[:], t.t12_t[:], t.t45_t[:])
                geng.tensor_add(t.s3, t.t12[:, :, 0:1], t.s3)       # alpha
                geng.tensor_sub(t.AD3, t.t12[:, :, 1:2], t.AD3)     # betaX

            def tail(t):
                nc.vector.tensor_mul(t.Ta, t.s3.broadcast_to([P, N, 4]), t.a4)
                nc.vector.tensor_mul(t.Tb, t.AD3.broadcast_to([P, N, 4]), t.b4)
                nc.vector.tensor_mul(t.G[:, :, 0:2],
                                     t.cm3.broadcast_to([P, N, 2]), t.wv)
                nc.vector.tensor_mul(t.G[:, :, 2:4],
                                     t.cu3.broadcast_to([P, N, 2]), t.wv)

            def act_copy(t):
                nc.scalar.activation(t.evens, t.odds, Copy)

            def gp_tail(t):
                geng.tensor_add(t.Ta_t[:], t.Ta_t[:], t.Tb_t[:])
                geng.tensor_add(t.Ta_t[:], t.Ta_t[:], t.G_t[:])

            def fin(t):
                nc.vector.scalar_tensor_tensor(t.S5[:, :, 0:1, 1], t.v3, -0.2,
                                               t.t3, mul, add)
                nc.vector.scalar_tensor_tensor(t.b4, t.b4, -0.2, t.Ta, mul, add)

            def store(t):
                nc.scalar.dma_start(ov[:, bass.ts(t.c, 10 * N)], t.S_t[:])

            phases = (front, gp_z2, acts, mid, gp_mid, tail, act_copy,
                      gp_tail, fin, store)
            if reps == 0:
                # fully unrolled: python loop over chunk pairs, no For_i
                for c in range(0, chunks, 2):
                    A = alloc(c)
                    B = alloc(c + 1)
                    for ph in phases:
                        ph(A)
                        ph(B)
            else:
                with tc.For_i(0, (chunks // 2) * reps, 1,
                              staggered_reset=staggered) as iv:
                    base = iv if reps == 1 else iv * 0
                    A = alloc(base * 2)
                    B = alloc(base * 2 + 1)
                    for ph in phases:
                        ph(A)
                        ph(B)

    orig = nc.to_json_bytes
    nc.to_json_bytes = lambda: _fix_bir_json(orig())
    return nc


def _build7(R: int, N: int, chunks: int = 12, s_bufs: int = 4, k_bufs: int = 3,
            sc_bufs: int = 3, reps: int = 1, staggered: bool = True,
            parts: str = "all", gp_on_dve: bool = True):
    """v7: unrolled 3-deep skewed software pipeline, fused-view op set.

    Per chunk: DVE 13 ops / 33N elems, GPSIMD 5 ops / 14N, ACT 5 ops / 9N.
    Multi-slot AP views fuse what v5 did in 28 ops into 23:
      XY4 = (k0,k2,k1,k3)*(w,w,v,v)            -> Z2 = halves sum (GP)
      SC slots: 0=P(t,h chain) 1=u 2=D 3=cm 4=cu 5=ns 6=AD
      CC2: (cm,cu) = A*(P,u) in one op; T4 = (cm,cm,cu,cu)*(k0,k1,k2,k3)
      t36 = T4 halves sum (GP); AB2 = t36 - (ns,AD) (GP) = (alpha,betaX)
      TT8 = (alpha*4,betaX*4)*(a1..a4,b1..b4); T8 = halves sum (GP)
      G = (cm,cm,cu,cu)*(w,v,w,v); S4 = T8+G (GP)
    Skew: slot t emits load(t), P1(t-1), P2(t-2), P3(t-3)+store(t-3), so
    each engine streams three different chunks' phases back to back.
    """
    import concourse.bass as bass
    import concourse.tile as tile
    import concourse.mybir as mybir
    from concourse.ap import AP

    F32 = mybir.dt.float32
    mul = mybir.AluOpType.mult
    add = mybir.AluOpType.add
    sub = mybir.AluOpType.subtract
    Copy = mybir.ActivationFunctionType.Copy
    Ln = mybir.ActivationFunctionType.Ln
    Exp = mybir.ActivationFunctionType.Exp

    nc = bass.Bass("TRN2")
    state_d = nc.dram_tensor("state", [R, 10], F32, kind="ExternalInput")
    k_d = nc.dram_tensor("K", [R, 4], F32, kind="ExternalInput")
    out_d = nc.dram_tensor("out", [R, 10], F32, kind="ExternalOutput")
    sv = state_d[:].rearrange("(p n) m -> p (n m)", p=P)
    kv = k_d[:].rearrange("(p n) m -> p (n m)", p=P)
    ov = out_d[:].rearrange("(p n) m -> p (n m)", p=P)

    def mkap(tile_ap, offset, dims):
        part = tile_ap.ap[0]
        return AP(tile_ap.tensor, offset, [list(part)] + [list(d) for d in dims])

    with tile.TileContext(nc) as tc:
        with (
            tc.tile_pool(name="sio", bufs=s_bufs) as sio,
            tc.tile_pool(name="kio", bufs=k_bufs) as kio,
            tc.tile_pool(name="scp", bufs=sc_bufs) as scp,
            tc.tile_pool(name="tmp", bufs=2) as tp,
            tc.tile_pool(name="pre", bufs=1) as pre,
        ):
            warm = pre.tile([P, 1], F32)
            b10 = pre.tile([P, 1], F32)
            nc.vector.memset(warm[:], 1.0)
            nc.vector.memset(b10[:], 10.0)
            nc.scalar.activation(warm[:], warm[:], Ln, bias=b10[:])
            nc.scalar.activation(warm[:], warm[:], Exp)

            class Tk:
                pass

            def load(c):
                t = Tk()
                t.c = c
                t.S_t = sio.tile([P, 10 * N], F32, tag="S")
                t.K_t = kio.tile([P, 4 * N], F32, tag="K")
                nc.sync.dma_start(t.S_t[:], sv[:, c * 10 * N:(c + 1) * 10 * N])
                nc.sync.dma_start(t.K_t[:], kv[:, c * 4 * N:(c + 1) * 4 * N])
                S5 = t.S_t[:].rearrange("p (n c two) -> p n c two", two=2, c=5)
                t.S5 = S5
                t.w3 = S5[:, :, 0:1, 0]
                t.v3 = S5[:, :, 0:1, 1]
                t.b4 = S5[:, :, 1:5, 1]
                t.evens = S5[:, :, :, 0]
                t.odds = S5[:, :, :, 1]
                # (w,w,v,v) ; (w,v,w,v) ; (a1..a4,b1..b4)
                t.wwvv = mkap(t.S_t[:], 0, [[10, N], [1, 2], [0, 2]])
                t.wvwv = mkap(t.S_t[:], 0, [[10, N], [0, 2], [1, 2]])
                t.aug8 = mkap(t.S_t[:], 2, [[10, N], [1, 2], [2, 4]])
                # (k0,k2,k1,k3) ; (k0,k1,k2,k3)
                t.k0213 = mkap(t.K_t[:], 0, [[4, N], [1, 2], [2, 2]])
                t.k4 = mkap(t.K_t[:], 0, [[4, N], [1, 4]])
                t.XY_t = tp.tile([P, 4 * N], F32, tag="XY")
                t.SC_t = scp.tile([P, 7 * N], F32, tag="SC")
                t.A_t = scp.tile([P, N], F32, tag="A")
                t.A4_t = scp.tile([P, N], F32, tag="A4")
                t.T4_t = tp.tile([P, 4 * N], F32, tag="T4")
                t.AB_t = tp.tile([P, 2 * N], F32, tag="AB")
                t.TT_t = tp.tile([P, 8 * N], F32, tag="TT")
                t.G_t = tp.tile([P, 4 * N], F32, tag="G")
                if parts != "dvegp":
                    return _finish_tile_views(t)
                t.Gp2a_t = tp.tile([P, 2 * N], F32, tag="Gp2a")
                t.Gp2b_t = tp.tile([P, 2 * N], F32, tag="Gp2b")
                t.Gp4a_t = tp.tile([P, 4 * N], F32, tag="Gp4a")
                t.Gp4b_t = tp.tile([P, 4 * N], F32, tag="Gp4b")
                t.Gp2a = t.Gp2a_t[:].rearrange("p (n c) -> p n c", c=2)
                t.Gp2b = t.Gp2b_t[:].rearrange("p (n c) -> p n c", c=2)
                t.Gp4a = t.Gp4a_t[:].rearrange("p (n c) -> p n c", c=4)
                t.Gp4b = t.Gp4b_t[:].rearrange("p (n c) -> p n c", c=4)
                return _finish_tile_views(t)

            def _finish_tile_views(t):
                t.XY = t.XY_t[:].rearrange("p (n c) -> p n c", c=4)
                sc = t.SC_t[:].rearrange("p (n c) -> p n c", c=7)
                t.P3s = sc[:, :, 0:1]
                t.u3 = sc[:, :, 1:2]
                t.D3 = sc[:, :, 2:3]
                t.cmcu = sc[:, :, 3:5]
                t.PU = sc[:, :, 0:2]
                t.ns3 = sc[:, :, 5:6]
                t.AD3 = sc[:, :, 6:7]
                t.nsAD = sc[:, :, 5:7]
                # (cm,cm,cu,cu)
                t.cc4 = mkap(t.SC_t[:], 3, [[7, N], [1, 2], [0, 2]])
                t.A3 = t.A_t[:].unsqueeze(2)
                t.A43 = t.A4_t[:].unsqueeze(2)
                t.T4 = t.T4_t[:].rearrange("p (n c) -> p n c", c=4)
                t.AB = t.AB_t[:].rearrange("p (n c) -> p n c", c=2)
                t.TT = t.TT_t[:].rearrange("p (n c) -> p n c", c=8)
                t.G = t.G_t[:].rearrange("p (n c) -> p n c", c=4)
                # (alpha*4, betaX*4)
                t.ab8 = mkap(t.AB_t[:], 0, [[2, N], [1, 2], [0, 4]])
                return t

            dve = parts in ("all", "dve", "dvegp", "dveact")
            gp = parts in ("all", "gp", "dvegp")
            act = parts in ("all", "act", "dveact")
            indep = parts == "dvegp"  # gp reads S-backed views: no dve<->gp deps
            geng = nc.vector if gp_on_dve else nc.gpsimd

            def p1(t):
                if dve:
                    nc.vector.tensor_mul(t.XY, t.k0213, t.wwvv)
                    nc.vector.scalar_tensor_tensor(t.u3, t.w3, 0.2, t.v3,
                                                   mul, add)
                if gp:
                    z_in0 = (t.XY[:, :, 0:2] if (dve and not indep)
                             else mkap(t.S_t[:], 0, [[10, N], [1, 2]]))
                    z_in1 = (t.XY[:, :, 2:4] if (dve and not indep)
                             else mkap(t.S_t[:], 2, [[10, N], [1, 2]]))
                    z_out = t.XY[:, :, 0:2] if not indep else t.Gp2a
                    geng.tensor_add(z_out, z_in0, z_in1)  # Z2
                if act:
                    mMv = t.XY[:, :, 0:1] if (dve or gp) else t.w3
                    mDv = t.XY[:, :, 1:2] if (dve or gp) else t.v3
                    nc.scalar.activation(t.A3, mMv, Ln, bias=b10[:],
                                         scale=-1.0)               # ln M
                    nc.scalar.activation(t.A3, t.A3, Exp, scale=-1.0)  # A=1/M
                    nc.scalar.activation(t.A43, t.A3, Copy, scale=-4.0)
                    nc.scalar.activation(t.D3, mDv, Copy, bias=1.0,
                                         scale=-1.0)               # D=1-mD

            def p2(t):
                if dve:
                    A3r = t.A3 if act else t.u3
                    A43r = t.A43 if act else t.u3
                    D3r = t.D3 if act else t.u3
                    nc.vector.tensor_mul(t.P3s, D3r, t.u3)          # t=D*u
                    nc.vector.scalar_tensor_tensor(t.P3s, t.w3, -4.0, t.P3s,
                                                   mul, sub)        # h=-4w-t
                    nc.vector.scalar_tensor_tensor(t.P3s, t.P3s, 0.02, A3r,
                                                   add, mul)        # P=(h+.02)A
                    nc.vector.tensor_mul(t.cmcu, A3r.broadcast_to([P, N, 2]),
                                         t.PU)                      # (cm,cu)
                    nc.vector.tensor_mul(t.AD3, A3r, D3r)           # AD
                    nc.vector.scalar_tensor_tensor(t.ns3, t.AD3, 0.2, A43r,
                                                   mul, sub)        # ns
                    nc.vector.tensor_mul(t.T4, t.cc4, t.k4)
                if gp:
                    t36_0 = (t.T4[:, :, 0:2] if (dve and not indep)
                             else mkap(t.S_t[:], 4, [[10, N], [1, 2]]))
                    t36_1 = (t.T4[:, :, 2:4] if (dve and not indep)
                             else mkap(t.S_t[:], 6, [[10, N], [1, 2]]))
                    t36_out = t.T4[:, :, 0:2] if not indep else t.Gp2a
                    geng.tensor_add(t36_out, t36_0, t36_1)
                    ns_in = (t.nsAD if (dve and not indep)
                             else mkap(t.S_t[:], 2, [[10, N], [1, 2]]))
                    ab_in0 = t.T4[:, :, 0:2] if not indep else t.Gp2a
                    ab_out = t.AB if not indep else t.Gp2b
                    geng.tensor_sub(ab_out, ab_in0, ns_in)

            def p3(t):
                if dve:
                    ABr = t.ab8 if (gp and not indep) else mkap(
                        t.SC_t[:], 5, [[7, N], [1, 2], [0, 4]])
                    nc.vector.tensor_mul(t.TT, ABr, t.aug8)
                    nc.vector.tensor_mul(t.G, t.cc4, t.wvwv)        # gamma
                if act:
                    nc.scalar.activation(t.evens, t.odds, Copy)
                if gp:
                    t8_in0 = (t.TT[:, :, 0:4] if (dve and not indep)
                              else mkap(t.S_t[:], 2, [[10, N], [2, 4]]))
                    t8_in1 = (t.TT[:, :, 4:8] if (dve and not indep)
                              else mkap(t.S_t[:], 3, [[10, N], [2, 4]]))
                    t8_out = t.TT[:, :, 0:4] if not indep else t.Gp4a
                    geng.tensor_add(t8_out, t8_in0, t8_in1)
                    s4_in = (t.G if (dve and not indep)
                             else mkap(t.S_t[:], 1, [[10, N], [2, 4]]))
                    s4_in0 = t.TT[:, :, 0:4] if not indep else t.Gp4a
                    s4_out = t.G if not indep else t.Gp4b
                    geng.tensor_add(s4_out, s4_in0, s4_in)   # S4
                if dve:
                    S4r = t.G
                    nc.vector.scalar_tensor_tensor(t.S5[:, :, 0:1, 1], t.v3,
                                                   -0.2, t.P3s, mul, add)  # f2
                    nc.vector.scalar_tensor_tensor(t.b4, t.b4, -0.2, S4r,
                                                   mul, add)
                nc.scalar.dma_start(ov[:, t.c * 10 * N:(t.c + 1) * 10 * N],
                                    t.S_t[:])

            def emit_all():
                live = []
                for c in range(chunks + 3):
                    if c < chunks:
                        live.append(load(c))
                    if c >= 1 and c - 1 < chunks:
                        p1(live[c - 1])
                    if c >= 2 and c - 2 < chunks:
                        p2(live[c - 2])
                    if c >= 3:
                        p3(live[c - 3])
                        live[c - 3] = None

            if reps == 1:
                emit_all()
            else:
                with tc.For_i(0, reps, 1, staggered_reset=staggered):
                    emit_all()

    orig = nc.to_json_bytes
    nc.to_json_bytes = lambda: _fix_bir_json(orig())
    return nc


def _build9(R: int, N: int, chunks: int = 12, reps: int = 1,
            lean_bufs: bool = False, fp16_tail: bool = False,
            t4_16: bool = True, s_bufs: int = 4, k_bufs: int = 3):
    """v9: A-factored alpha/beta (all elementwise on DVE, ACT non-contending).

    Per row:  mM,mD = K·(w,v) pairs ; A = 1/(10-mM) via ACT ln/exp
       D = 1-mD ; ns = 4+0.2D = 4.2-0.2mD   (both pure ACT affines of mD)
       u = 0.2w+v ; P = (0.02 - 4w - D u) A
       alpha~ = P k0 + u k2 - ns ; beta~ = P k1 + u k3 - D
       df_p = A (alpha~ a_p + beta~ b_p + gamma~_p) - 0.2 b_p,
       gamma~ = (Pw, Pv, uw, uv) ;  f2 = P - 0.2v
    DVE 16 ops / 47N per chunk; ACT 5 ops / 9N. 3-deep skewed pipeline,
    stores on ACT HWDGE ring.
    """
    import concourse.bass as bass
    import concourse.tile as tile
    import concourse.mybir as mybir
    from concourse.ap import AP

    F32 = mybir.dt.float32
    mul = mybir.AluOpType.mult
    add = mybir.AluOpType.add
    sub = mybir.AluOpType.subtract
    Copy = mybir.ActivationFunctionType.Copy
    Ln = mybir.ActivationFunctionType.Ln
    Exp = mybir.ActivationFunctionType.Exp

    nc = bass.Bass("TRN2")
    state_d = nc.dram_tensor("state", [R, 10], F32, kind="ExternalInput")
    k_d = nc.dram_tensor("K", [R, 4], F32, kind="ExternalInput")
    out_d = nc.dram_tensor("out", [R, 10], F32, kind="ExternalOutput")
    sv = state_d[:].rearrange("(p n) m -> p (n m)", p=P)
    kv = k_d[:].rearrange("(p n) m -> p (n m)", p=P)
    ov = out_d[:].rearrange("(p n) m -> p (n m)", p=P)

    def mkap(tile_ap, offset, dims):
        part = tile_ap.ap[0]
        return AP(tile_ap.tensor, offset, [list(part)] + [list(d) for d in dims])

    with tile.TileContext(nc) as tc:
        with (
            tc.tile_pool(name="sio", bufs=s_bufs) as sio,
            tc.tile_pool(name="kio", bufs=k_bufs) as kio,
            tc.tile_pool(name="scp", bufs=3) as scp,
            tc.tile_pool(name="tmp", bufs=2) as tp,
            tc.tile_pool(name="pre", bufs=1) as pre,
        ):
            warm = pre.tile([P, 1], F32)
            b10 = pre.tile([P, 1], F32)
            nc.vector.memset(warm[:], 1.0)
            nc.vector.memset(b10[:], 10.0)
            nc.scalar.activation(warm[:], warm[:], Ln, bias=b10[:])
            nc.scalar.activation(warm[:], warm[:], Exp)

            class Tk:
                pass

            def load(c):
                t = Tk()
                t.c = c
                t.S_t = sio.tile([P, 10 * N], F32, tag="S")
                t.K_t = kio.tile([P, 4 * N], F32, tag="K")
                nc.sync.dma_start(t.S_t[:], sv[:, c * 10 * N:(c + 1) * 10 * N])
                nc.sync.dma_start(t.K_t[:], kv[:, c * 4 * N:(c + 1) * 4 * N])
                S5 = t.S_t[:].rearrange("p (n c two) -> p n c two", two=2, c=5)
                t.S5 = S5
                t.w3 = S5[:, :, 0:1, 0]
                t.v3 = S5[:, :, 0:1, 1]
                t.b4 = S5[:, :, 1:5, 1]
                t.evens = S5[:, :, :, 0]
                t.odds = S5[:, :, :, 1]
                t.wwvv = mkap(t.S_t[:], 0, [[10, N], [1, 2], [0, 2]])
                t.wvwv = mkap(t.S_t[:], 0, [[10, N], [0, 2], [1, 2]])
                t.aug8 = mkap(t.S_t[:], 2, [[10, N], [1, 2], [2, 4]])
                t.k0213 = mkap(t.K_t[:], 0, [[4, N], [1, 2], [2, 2]])
                t.k4 = mkap(t.K_t[:], 0, [[4, N], [1, 4]])

                TB = 1 if lean_bufs else 2
                F16 = mybir.dt.bfloat16
                FT = F16 if fp16_tail else F32
                t.XY_t = tp.tile([P, 4 * N], F32, tag="XY")
                t.SC_t = scp.tile([P, 6 * N], F32, tag="SC")
                t.A_t = scp.tile([P, N], F32, tag="A")
                t.T4_t = tp.tile([P, 4 * N], FT if t4_16 else F32,
                                 tag="T4", bufs=TB)
                t.TT_t = tp.tile([P, 8 * N], FT, tag="TT", bufs=TB)
                t.G_t = tp.tile([P, 4 * N], FT, tag="G", bufs=TB)
                t.AS_t = tp.tile([P, 4 * N], F32, tag="AS", bufs=TB)

                t.XY = t.XY_t[:].rearrange("p (n c) -> p n c", c=4)
                sc = t.SC_t[:].rearrange("p (n c) -> p n c", c=6)
                t.P3s = sc[:, :, 0:1]     # t, h, P chain
                t.u3 = sc[:, :, 1:2]
                t.ns3 = sc[:, :, 2:3]     # ACT: 4.2 - 0.2 mD
                t.D3 = sc[:, :, 3:4]      # ACT: 1 - mD
                t.nsD = sc[:, :, 2:4]
                t.AB = sc[:, :, 4:6]      # alpha~, beta~
                t.Pu4 = mkap(t.SC_t[:], 0, [[6, N], [1, 2], [0, 2]])  # P,P,u,u
                t.ab8 = mkap(t.SC_t[:], 4, [[6, N], [1, 2], [0, 4]])  # al*4,be*4
                t.A3 = t.A_t[:].unsqueeze(2)
                t.T4 = t.T4_t[:].rearrange("p (n c) -> p n c", c=4)
                t.TT = t.TT_t[:].rearrange("p (n c) -> p n c", c=8)
                t.G = t.G_t[:].rearrange("p (n c) -> p n c", c=4)
                t.AS = t.AS_t[:].rearrange("p (n c) -> p n c", c=4)
                return t

            def p1(t):
                nc.vector.tensor_mul(t.XY, t.k0213, t.wwvv)
                nc.vector.scalar_tensor_tensor(t.u3, t.w3, 0.2, t.v3, mul, add)
                nc.vector.tensor_add(t.XY[:, :, 0:2], t.XY[:, :, 0:2],
                                     t.XY[:, :, 2:4])            # (mM,mD)
                nc.scalar.activation(t.A3, t.XY[:, :, 0:1], Ln, bias=b10[:],
                                     scale=-1.0)
                nc.scalar.activation(t.A3, t.A3, Exp, scale=-1.0)   # A=1/M
                nc.scalar.activation(t.ns3, t.XY[:, :, 1:2], Copy, bias=4.2,
                                     scale=-0.2)                 # ns=4+0.2D
                nc.scalar.activation(t.D3, t.XY[:, :, 1:2], Copy, bias=1.0,
                                     scale=-1.0)                 # D=1-mD

            def p2(t):
                nc.vector.tensor_mul(t.P3s, t.D3, t.u3)              # t=D*u
                nc.vector.scalar_tensor_tensor(t.P3s, t.w3, -4.0, t.P3s,
                                               mul, sub)             # h=-4w-t
                nc.vector.scalar_tensor_tensor(t.P3s, t.P3s, 0.02, t.A3,
                                               add, mul)             # P
                nc.vector.tensor_mul(t.T4, t.Pu4, t.k4)  # Pk0,Pk1,uk2,uk3
                nc.vector.tensor_add(t.T4[:, :, 0:2], t.T4[:, :, 0:2],
                                     t.T4[:, :, 2:4])     # (Pk0+uk2, Pk1+uk3)
                nc.vector.tensor_sub(t.AB, t.T4[:, :, 0:2], t.nsD)  # al~,be~

            def p3(t):
                nc.vector.tensor_mul(t.TT, t.ab8, t.aug8)
                nc.vector.tensor_mul(t.G, t.Pu4, t.wvwv)    # Pw,Pv,uw,uv
                nc.scalar.activation(t.evens, t.odds, Copy)
                nc.vector.tensor_add(t.TT[:, :, 0:4], t.TT[:, :, 0:4],
                                     t.TT[:, :, 4:8])               # T8
                nc.vector.tensor_add(t.G, t.TT[:, :, 0:4], t.G)     # S4
                nc.vector.tensor_mul(t.AS, t.A3.broadcast_to([P, N, 4]), t.G)
                nc.vector.scalar_tensor_tensor(t.S5[:, :, 0:1, 1], t.v3, -0.2,
                                               t.P3s, mul, add)     # f2
                nc.vector.scalar_tensor_tensor(t.b4, t.b4, -0.2, t.AS, mul, add)
                nc.scalar.dma_start(ov[:, t.c * 10 * N:(t.c + 1) * 10 * N],
                                    t.S_t[:])

            def emit_all():
                live = {}
                for c in range(chunks + 3):
                    if c < chunks:
                        live[c] = load(c)
                    if c >= 1 and c - 1 < chunks:
                        p1(live[c - 1])
                    if c >= 2 and c - 2 < chunks:
                        p2(live[c - 2])
                    if c >= 3:
                        p3(live.pop(c - 3))

            if reps == 1:
                emit_all()
            else:
                with tc.For_i(0, reps, 1, staggered_reset=True):
                    emit_all()

    orig = nc.to_json_bytes
    nc.to_json_bytes = lambda: _fix_bir_json(orig())
    return nc


V3_CHUNKS = 6
V4_CHUNKS = 8
V5_CHUNKS = 10
V7_CHUNKS = 12
V9_CHUNKS = 12
KERNEL_VERSION = 9


def _get_program(B: int, reps: int = 1):
    key = (B, reps, KERNEL_VERSION)
    if key not in _CACHE:
        if KERNEL_VERSION == 9:
            N = -(-B // (N_CORES * P * V9_CHUNKS))  # ceil
            R = P * V9_CHUNKS * N
            _CACHE[key] = (_build9(R, N, V9_CHUNKS, reps=reps,
                                   fp16_tail=True), R)
        elif KERNEL_VERSION == 5:
            N = -(-B // (N_CORES * P * V5_CHUNKS))  # ceil
            R = P * V5_CHUNKS * N
            _CACHE[key] = (_build5(R, N, reps, V5_CHUNKS), R)
        elif KERNEL_VERSION == 4:
            N = -(-B // (N_CORES * P * V4_CHUNKS))  # ceil
            R = P * V4_CHUNKS * N
            _CACHE[key] = (_build4(R, N, reps, V4_CHUNKS), R)
        else:
            N = -(-B // (N_CORES * P * V3_CHUNKS))  # ceil
            R = P * V3_CHUNKS * N
            _CACHE[key] = (_build3(R, N, reps, V3_CHUNKS), R)
    return _CACHE[key]


def _run(state: np.ndarray, K: np.ndarray, trace: bool = False, reps: int = 1):
    from concourse import bass_utils

    B = state.shape[0]
    nc, R = _get_program(B, reps)
    BP = N_CORES * R

    state_p = np.zeros((BP, 10), dtype=np.float32)
    state_p[:B] = state
    k_p = np.zeros((BP, 4), dtype=np.float32)
    k_p[:B] = K

    in_maps = [
        {"state": state_p[i * R:(i + 1) * R], "K": k_p[i * R:(i + 1) * R]}
        for i in range(N_CORES)
    ]
    res = bass_utils.run_bass_kernel_spmd(
        nc, in_maps, core_ids=list(range(N_CORES)), trace=trace
    )
    out = np.concatenate([r["out"] for r in res.results], axis=0)[:B]
    return out, res


def kernel(t, state, K):
    state = np.ascontiguousarray(np.asarray(state), dtype=np.float32)
    K = np.ascontiguousarray(np.asarray(K), dtype=np.float32)
    out, _ = _run(state, K, trace=False)
    return out

